# revision 6
# baseline (speedup 1.0000x reference)
"""PolynormerAttention TRN2 kernel.

Math (reference):
  h   = x @ h_w + h_b                      [N, 512]
  k   = sigmoid(x @ k_w)  (q = k)          [N, 512] viewed [N, D=64, H=8], j = d*8+h
  v   = x @ v_w                            [N, 512]
  kv  = einsum('ndh,nmh->dmh', k, v)       -> masked K^T V  ("KVB"[j, j'] with j%8==j'%8)
  num = q @ KVB                            [N, 512]
  den = q @ KS          (KS[j, h'] = ksum[j] * (j%8==h'))   [N, 8]
  attn= num / den (per head)
  y   = relu((LN(attn) * (h + 0.9)) @ out_w + out_b)

Sharding: data-parallel on N across 8 cores; AllReduce of [K^T V | ksum]
([513, 512] fp32, ~1 MB) between pass 1 and pass 2 inside a single launch.

Precision: fp32r (TF32-like, full PE rate at free-dim >= 256) for all matmuls,
fp32 PSUM accumulation.
"""

import math

import numpy as np

import concourse.bass as bass
import concourse.bacc as bacc
import concourse.tile as tile
from concourse import mybir
from concourse.bass_utils import run_bass_kernel_spmd
from concourse.masks import make_identity

N_CORES = 8
N_FULL = 100000
C = 512
H = 8
D = 64
INNER = 512
BETA = 0.9
EPS = 1e-5
P = 128
NB = 256  # rows per block (2 subtiles of 128)
F32 = mybir.dt.float32
F32R = mybir.dt.float32r

N_SHARD = N_FULL // N_CORES          # 12500
N_PAD = ((N_SHARD + NB - 1) // NB) * NB  # 12544


def _bcast_free(ap, reps, at=-1):
    """Repeat the innermost free dim `reps` times via a stride-0 AP dim."""
    new = [list(d) for d in ap.ap]
    new.insert(len(new) - 1 if at == -1 else at, [0, reps])
    return bass.AP(tensor=ap.tensor, offset=ap.offset, ap=new)


def build_nc(n_pad=N_PAD, n_real=N_SHARD, n_cores=N_CORES, apply_ln_affine=False):
    assert n_pad % NB == 0
    nblocks = n_pad // NB
    # padded rows contribute sigmoid(0)=0.5 to every ksum entry
    ks_corr = float(n_cores * (n_pad - n_real) * 0.5)

    nc = bacc.Bacc("TRN2", target_bir_lowering=False, debug=False,
                   num_devices=n_cores)

    x_d = nc.dram_tensor("x", [n_pad, C], F32, kind="ExternalInput").ap()
    wh_d = nc.dram_tensor("h_w", [C, INNER], F32, kind="ExternalInput").ap()
    wk_d = nc.dram_tensor("k_w", [C, INNER], F32, kind="ExternalInput").ap()
    wv_d = nc.dram_tensor("v_w", [C, INNER], F32, kind="ExternalInput").ap()
    wo_d = nc.dram_tensor("out_w", [INNER, INNER], F32, kind="ExternalInput").ap()
    hb_d = nc.dram_tensor("hb_beta", [1, INNER], F32, kind="ExternalInput").ap()
    ob_d = nc.dram_tensor("ob_big", [P, INNER], F32, kind="ExternalInput").ap()
    mask_d = nc.dram_tensor("mask_big", [P, INNER], F32, kind="ExternalInput").ap()
    if apply_ln_affine:
        g_d = nc.dram_tensor("g_big", [P, INNER], F32, kind="ExternalInput").ap()
        b_d = nc.dram_tensor("b_big", [P, INNER], F32, kind="ExternalInput").ap()
    y_d = nc.dram_tensor("y", [n_pad, INNER], F32, kind="ExternalOutput").ap()

    cc_in = nc.dram_tensor("cc_in", [INNER + 1, INNER], F32).ap()
    cc_out = nc.dram_tensor("cc_out", [INNER + 1, INNER], F32,
                            addr_space="Shared").ap()

    NCH = C // P  # 4 contraction chunks

    with tile.TileContext(nc) as tc:
        with (
            tc.tile_pool(name="consts", bufs=1) as consts,
            tc.tile_pool(name="sb1", bufs=3) as sb1,
            tc.tile_pool(name="kvsb", bufs=2) as kvsb,
            tc.tile_pool(name="sb2", bufs=2) as sb2,
        ):
            # ---------------- constants ----------------
            ident_f = consts.tile([P, P], F32)
            make_identity(nc, ident_f)
            ident = consts.tile([P, P], F32R)
            nc.scalar.copy(out=ident, in_=ident_f)
            ones_f = consts.tile([P, P], F32)
            nc.vector.memset(ones_f, 1.0)
            ones_p = consts.tile([P, 1], F32R)
            nc.scalar.copy(out=ones_p, in_=ones_f[:, 0:1])
            ones_1 = consts.tile([1, P], F32R)
            nc.scalar.copy(out=ones_1, in_=ones_f[0:1, :])
            eps_t = consts.tile([P, 1], F32)
            nc.vector.memset(eps_t, EPS)

            def load_w(name, dram):
                t = consts.tile([P, NCH, INNER], F32R, tag=name)
                nc.sync.dma_start(
                    out=t,
                    in_=dram.rearrange("(co ci) j -> ci co j", ci=P).bitcast(F32R),
                )
                return t

            wh_s = load_w("wh", wh_d)
            wk_s = load_w("wk", wk_d)
            wv_s = load_w("wv", wv_d)
            wo_s = load_w("wo", wo_d)
            hb_s = consts.tile([1, INNER], F32R)
            nc.sync.dma_start(out=hb_s, in_=hb_d.bitcast(F32R))
            ob_s = consts.tile([P, INNER], F32)
            nc.sync.dma_start(out=ob_s, in_=ob_d)
            mask_s = consts.tile([P, INNER], F32)
            nc.sync.dma_start(out=mask_s, in_=mask_d)
            if apply_ln_affine:
                g_s = consts.tile([P, INNER], F32)
                nc.sync.dma_start(out=g_s, in_=g_d)
                b_s = consts.tile([P, INNER], F32)
                nc.sync.dma_start(out=b_s, in_=b_d)

            # ================ PASS 1: kv + ksum ================
            with (
                tc.tile_pool(name="p1acc", bufs=1, space="PSUM") as p1acc,
                tc.tile_pool(name="p1work", bufs=1, space="PSUM") as p1work,
            ):
                ktv_ps = [p1acc.tile([P, INNER], F32, tag=f"ktv{c}",
                                     name=f"ktv{c}")
                          for c in range(NCH)]
                ks_ps = p1acc.tile([1, INNER], F32, tag="ks")

                n_sub = n_pad // P
                for blk in range(nblocks):
                    xt = sb1.tile([P, 2, C], F32R, tag="x")
                    nc.sync.dma_start(
                        out=xt,
                        in_=x_d[blk * NB:(blk + 1) * NB, :]
                        .rearrange("(s p) c -> p s c", p=P).bitcast(F32R),
                    )
                    for sub in range(2):
                        si = blk * 2 + sub
                        xT = sb1.tile([P, NCH, P], F32R, tag="xT")
                        for c in range(NCH):
                            tp = p1work.tile([P, P], F32R, tag="xTt")
                            nc.tensor.transpose(
                                tp, xt[:, sub, c * P:(c + 1) * P], ident)
                            nc.scalar.copy(out=xT[:, c, :], in_=tp)
                        # K = sigmoid(x @ k_w)
                        kp = p1work.tile([P, INNER], F32, tag="K")
                        for c in range(NCH):
                            nc.tensor.matmul(kp, xT[:, c, :], wk_s[:, c, :],
                                             start=(c == 0), stop=(c == NCH - 1))
                        ksb = sb1.tile([P, INNER], F32R, tag="Ksb")
                        nc.scalar.activation(
                            out=ksb, in_=kp,
                            func=mybir.ActivationFunctionType.Sigmoid)
                        # V = x @ v_w
                        vp = p1work.tile([P, INNER], F32, tag="V")
                        for c in range(NCH):
                            nc.tensor.matmul(vp, xT[:, c, :], wv_s[:, c, :],
                                             start=(c == 0), stop=(c == NCH - 1))
                        vsb = sb1.tile([P, INNER], F32R, tag="Vsb")
                        nc.scalar.copy(out=vsb, in_=vp)
                        # K^T V accumulation (full [512, 512], masked later)
                        for cm in range(NCH):
                            nc.tensor.matmul(
                                ktv_ps[cm], ksb[:, cm * P:(cm + 1) * P], vsb,
                                start=(si == 0), stop=(si == n_sub - 1))
                        # ksum accumulation
                        nc.tensor.matmul(ks_ps, ones_p, ksb,
                                         start=(si == 0), stop=(si == n_sub - 1))

                for c in range(NCH):
                    kvstage = sb1.tile([P, INNER], F32, tag="kvstage")
                    nc.scalar.copy(out=kvstage, in_=ktv_ps[c])
                    nc.sync.dma_start(out=cc_in[c * P:(c + 1) * P, :],
                                      in_=kvstage)
                ksstage = sb1.tile([1, INNER], F32, tag="ksstage")
                nc.scalar.copy(out=ksstage, in_=ks_ps)
                nc.sync.dma_start(out=cc_in[INNER:INNER + 1, :], in_=ksstage)

            # ================ AllReduce ================
            nc.gpsimd.collective_compute(
                "AllReduce",
                mybir.AluOpType.add,
                replica_groups=[list(range(n_cores))],
                ins=[cc_in[:, :]],
                outs=[cc_out[:, :]],
            )

            # ---- build KVB (masked kv) and KS from reduced buffer ----
            kvb = kvsb.tile([P, NCH, INNER], F32R, tag="kvb")
            kvraw = kvsb.tile([P, NCH, INNER], F32, tag="kvraw")
            nc.sync.dma_start(
                out=kvraw,
                in_=cc_out[0:INNER, :].rearrange("(co ci) j -> ci co j", ci=P))
            for c in range(NCH):
                nc.vector.tensor_mul(kvb[:, c, :], kvraw[:, c, :], mask_s)
            kst = kvsb.tile([P, NCH], F32, tag="kst")
            nc.sync.dma_start(
                out=kst,
                in_=cc_out[INNER, :].rearrange("(co ci) -> ci co", ci=P))
            kst2 = kvsb.tile([P, NCH], F32, tag="kst2")
            nc.vector.tensor_scalar_add(kst2, kst, -ks_corr)
            ks_s = kvsb.tile([P, NCH, H], F32R, tag="ks_s")
            for c in range(NCH):
                nc.vector.tensor_scalar_mul(ks_s[:, c, :], mask_s[:, 0:H],
                                            kst2[:, c:c + 1])

            # ================ PASS 2 ================
            with (
                tc.tile_pool(name="p2a", bufs=2, space="PSUM") as p2a,
                tc.tile_pool(name="p2b", bufs=1, space="PSUM") as p2b,
            ):
                for blk in range(nblocks):
                    xt = sb2.tile([P, 2, C], F32R, tag="x2")
                    nc.sync.dma_start(
                        out=xt,
                        in_=x_d[blk * NB:(blk + 1) * NB, :]
                        .rearrange("(s p) c -> p s c", p=P).bitcast(F32R),
                    )
                    xT = sb2.tile([P, NCH, NB], F32R, tag="xT2")
                    for sub in range(2):
                        for c in range(NCH):
                            tp = p2a.tile([P, P], F32R, tag="tr")
                            nc.tensor.transpose(
                                tp, xt[:, sub, c * P:(c + 1) * P], ident)
                            nc.scalar.copy(
                                out=xT[:, c, sub * P:(sub + 1) * P], in_=tp)
                    # QT[jc] = sigmoid(k_w^T x^T) : [128 j, 256 n]
                    qt = sb2.tile([P, NCH, NB], F32R, tag="qt")
                    for jc in range(NCH):
                        qp = p2a.tile([P, NB], F32, tag="qt")
                        for c in range(NCH):
                            nc.tensor.matmul(
                                qp, wk_s[:, c, jc * P:(jc + 1) * P], xT[:, c, :],
                                start=(c == 0), stop=(c == NCH - 1))
                        nc.scalar.activation(
                            out=qt[:, jc, :], in_=qp,
                            func=mybir.ActivationFunctionType.Sigmoid)

                    for sub in range(2):
                        row0 = blk * NB + sub * P
                        # h + h_b + beta  (bias via rank-1 ones matmul)
                        hp = p2b.tile([P, INNER], F32, tag="h")
                        for c in range(NCH):
                            nc.tensor.matmul(
                                hp, xT[:, c, sub * P:(sub + 1) * P],
                                wh_s[:, c, :], start=(c == 0), stop=False)
                        nc.tensor.matmul(hp, ones_1, hb_s,
                                         start=False, stop=True)
                        # num / den
                        nump = p2b.tile([P, INNER], F32, tag="num")
                        for c in range(NCH):
                            nc.tensor.matmul(
                                nump, qt[:, c, sub * P:(sub + 1) * P],
                                kvb[:, c, :], start=(c == 0), stop=(c == NCH - 1))
                        denp = p2b.tile([P, H], F32, tag="den")
                        for c in range(NCH):
                            nc.tensor.matmul(
                                denp, qt[:, c, sub * P:(sub + 1) * P],
                                ks_s[:, c, :], start=(c == 0), stop=(c == NCH - 1))
                        rec = sb2.tile([P, H], F32, tag="rec")
                        nc.vector.reciprocal(rec, denp)
                        attn = sb2.tile([P, INNER], F32, tag="attn")
                        nc.vector.tensor_mul(attn, nump,
                                             _bcast_free(rec[:, :], D))
                        # layernorm stats
                        st = sb2.tile([P, 6], F32, tag="st")
                        nc.vector.bn_stats(out=st, in_=attn)
                        mv = sb2.tile([P, 2], F32, tag="mv")
                        nc.vector.bn_aggr(out=mv, in_=st)
                        rstd = sb2.tile([P, 1], F32, tag="rstd")
                        nc.scalar.activation(
                            out=rstd, in_=mv[:, 1:2],
                            func=mybir.ActivationFunctionType.Sqrt,
                            bias=eps_t, scale=1.0)
                        nc.vector.reciprocal(rstd, rstd)
                        ln = sb2.tile([P, INNER], F32, tag="ln")
                        nc.vector.tensor_scalar(
                            out=ln, in0=attn, scalar1=mv[:, 0:1], scalar2=rstd,
                            op0=mybir.AluOpType.subtract,
                            op1=mybir.AluOpType.mult)
                        if apply_ln_affine:
                            nc.vector.tensor_mul(ln, ln, g_s)
                            nc.vector.tensor_add(ln, ln, b_s)
                        z = sb2.tile([P, INNER], F32R, tag="z")
                        nc.vector.tensor_mul(z, ln, hp)
                        # z^T then y = relu(z @ out_w + out_b)
                        zT = sb2.tile([P, NCH, P], F32R, tag="zT")
                        for c in range(NCH):
                            tp = p2a.tile([P, P], F32R, tag="tr")
                            nc.tensor.transpose(tp, z[:, c * P:(c + 1) * P],
                                                ident)
                            nc.scalar.copy(out=zT[:, c, :], in_=tp)
                        yp = p2b.tile([P, INNER], F32, tag="y")
                        for c in range(NCH):
                            nc.tensor.matmul(yp, zT[:, c, :], wo_s[:, c, :],
                                             start=(c == 0), stop=(c == NCH - 1))
                        ysb = sb2.tile([P, INNER], F32, tag="ysb")
                        nc.vector.tensor_add(ysb, yp, ob_s)
                        nc.scalar.activation(
                            out=ysb, in_=ysb,
                            func=mybir.ActivationFunctionType.Relu)
                        nc.sync.dma_start(out=y_d[row0:row0 + P, :], in_=ysb)

    nc.compile()
    return nc


_cache = {}


def _get_nc(key, **kw):
    if key not in _cache:
        _cache[key] = build_nc(**kw)
    return _cache[key]


def make_in_maps(x, h_w, h_b, k_w, v_w, ln_g, ln_b, out_w, out_b,
                 n_pad=N_PAD, n_real=N_SHARD, n_cores=N_CORES):
    x = np.asarray(x, np.float32)
    n = x.shape[0]
    assert n == n_real * n_cores
    xp = np.zeros((n_cores, n_pad, C), np.float32)
    xp[:, :n_real, :] = x.reshape(n_cores, n_real, C)
    hb_beta = (np.asarray(h_b, np.float32) + BETA).reshape(1, INNER)
    ob_big = np.tile(np.asarray(out_b, np.float32).reshape(1, INNER), (P, 1))
    mask_big = (np.arange(P)[:, None] % H == np.arange(INNER)[None, :] % H
                ).astype(np.float32)
    common = dict(h_w=np.asarray(h_w, np.float32),
                  k_w=np.asarray(k_w, np.float32),
                  v_w=np.asarray(v_w, np.float32),
                  out_w=np.asarray(out_w, np.float32),
                  hb_beta=hb_beta, ob_big=ob_big, mask_big=mask_big)
    apply_affine = not (np.all(np.asarray(ln_g) == 1.0)
                        and np.all(np.asarray(ln_b) == 0.0))
    if apply_affine:
        common["g_big"] = np.tile(np.asarray(ln_g, np.float32).reshape(1, INNER),
                                  (P, 1))
        common["b_big"] = np.tile(np.asarray(ln_b, np.float32).reshape(1, INNER),
                                  (P, 1))
    return [dict(common, x=xp[i]) for i in range(n_cores)], apply_affine


def kernel(x, h_w, h_b, k_w, v_w, ln_g, ln_b, out_w, out_b):
    in_maps, apply_affine = make_in_maps(
        x, h_w, h_b, k_w, v_w, ln_g, ln_b, out_w, out_b)
    nc = _get_nc(("full", apply_affine), apply_ln_affine=apply_affine)
    res = run_bass_kernel_spmd(nc, in_maps, list(range(N_CORES)))
    y = np.concatenate([res.results[i]["y"][:N_SHARD] for i in range(N_CORES)],
                       axis=0)
    return y.astype(np.float32)


# revision 7
# speedup vs baseline: 1.1829x; 1.1829x over previous
"""PolynormerAttention TRN2 kernel.

Math (reference):
  h   = x @ h_w + h_b                      [N, 512]
  k   = sigmoid(x @ k_w)  (q = k)          [N, 512] viewed [N, D=64, H=8], j = d*8+h
  v   = x @ v_w                            [N, 512]
  kv  = einsum('ndh,nmh->dmh', k, v)       -> masked K^T V  ("KVB"[j, j'] with j%8==j'%8)
  num = q @ KVB                            [N, 512]
  den = q @ KS          (KS[j, h'] = ksum[j] * (j%8==h'))   [N, 8]
  attn= num / den (per head)
  y   = relu((LN(attn) * (h + 0.9)) @ out_w + out_b)

Sharding: data-parallel on N across 8 cores; AllReduce of [K^T V | ksum]
([513, 512] fp32, ~1 MB) between pass 1 and pass 2 inside a single launch.

Precision: fp32r (TF32-like, full PE rate at free-dim >= 256) for all matmuls,
fp32 PSUM accumulation.
"""

import math

import numpy as np

import concourse.bass as bass
import concourse.bacc as bacc
import concourse.tile as tile
from concourse import mybir
from concourse.bass_utils import run_bass_kernel_spmd
from concourse.masks import make_identity

N_CORES = 8
N_FULL = 100000
C = 512
H = 8
D = 64
INNER = 512
BETA = 0.9
EPS = 1e-5
P = 128
NB = 256  # rows per block (2 subtiles of 128)
F32 = mybir.dt.float32
F32R = mybir.dt.float32r

N_SHARD = N_FULL // N_CORES          # 12500
N_PAD = ((N_SHARD + NB - 1) // NB) * NB  # 12544


def _bcast_free(ap, reps, at=-1):
    """Repeat the innermost free dim `reps` times via a stride-0 AP dim."""
    new = [list(d) for d in ap.ap]
    new.insert(len(new) - 1 if at == -1 else at, [0, reps])
    return bass.AP(tensor=ap.tensor, offset=ap.offset, ap=new)


def build_nc(n_pad=N_PAD, n_real=N_SHARD, n_cores=N_CORES, apply_ln_affine=False,
             use_collective=True):
    assert n_pad % NB == 0
    nblocks = n_pad // NB
    # padded rows contribute sigmoid(0)=0.5 to every ksum entry
    ks_corr = float(n_cores * (n_pad - n_real) * 0.5)

    nc = bacc.Bacc("TRN2", target_bir_lowering=False, debug=False,
                   num_devices=n_cores)

    x_d = nc.dram_tensor("x", [n_pad, C], F32, kind="ExternalInput").ap()
    wh_d = nc.dram_tensor("h_w", [C, INNER], F32, kind="ExternalInput").ap()
    wk_d = nc.dram_tensor("k_w", [C, INNER], F32, kind="ExternalInput").ap()
    wv_d = nc.dram_tensor("v_w", [C, INNER], F32, kind="ExternalInput").ap()
    wo_d = nc.dram_tensor("out_w", [INNER, INNER], F32, kind="ExternalInput").ap()
    hb_d = nc.dram_tensor("hb_beta", [1, INNER], F32, kind="ExternalInput").ap()
    ob_d = nc.dram_tensor("ob_big", [P, INNER], F32, kind="ExternalInput").ap()
    mask_d = nc.dram_tensor("mask_big", [P, INNER], F32, kind="ExternalInput").ap()
    if apply_ln_affine:
        g_d = nc.dram_tensor("g_big", [P, INNER], F32, kind="ExternalInput").ap()
        b_d = nc.dram_tensor("b_big", [P, INNER], F32, kind="ExternalInput").ap()
    y_d = nc.dram_tensor("y", [n_pad, INNER], F32, kind="ExternalOutput").ap()

    cc_in = nc.dram_tensor("cc_in", [INNER + 1, INNER], F32).ap()
    cc_out = nc.dram_tensor("cc_out", [INNER + 1, INNER], F32,
                            addr_space="Shared").ap()

    NCH = C // P  # 4 contraction chunks

    with tile.TileContext(nc) as tc:
        with (
            tc.tile_pool(name="consts", bufs=1) as consts,
            tc.tile_pool(name="sb1", bufs=3) as sb1,
            tc.tile_pool(name="kvsb", bufs=2) as kvsb,
            tc.tile_pool(name="sb2", bufs=2) as sb2,
        ):
            # ---------------- constants ----------------
            ident_f = consts.tile([P, P], F32)
            make_identity(nc, ident_f)
            ident = consts.tile([P, P], F32R)
            nc.scalar.copy(out=ident, in_=ident_f)
            ones_f = consts.tile([P, P], F32)
            nc.vector.memset(ones_f, 1.0)
            ones_p = consts.tile([P, 1], F32R)
            nc.scalar.copy(out=ones_p, in_=ones_f[:, 0:1])
            ones_1 = consts.tile([1, P], F32R)
            nc.scalar.copy(out=ones_1, in_=ones_f[0:1, :])
            eps_t = consts.tile([P, 1], F32)
            nc.vector.memset(eps_t, EPS)

            def load_w(name, dram):
                t = consts.tile([P, NCH, INNER], F32R, tag=name)
                nc.sync.dma_start(
                    out=t,
                    in_=dram.rearrange("(co ci) j -> ci co j", ci=P).bitcast(F32R),
                )
                return t

            wh_s = load_w("wh", wh_d)
            wk_s = load_w("wk", wk_d)
            wv_s = load_w("wv", wv_d)
            wo_s = load_w("wo", wo_d)
            hb_s = consts.tile([1, INNER], F32R)
            nc.sync.dma_start(out=hb_s, in_=hb_d.bitcast(F32R))
            ob_s = consts.tile([P, INNER], F32)
            nc.sync.dma_start(out=ob_s, in_=ob_d)
            mask_s = consts.tile([P, INNER], F32)
            nc.sync.dma_start(out=mask_s, in_=mask_d)
            if apply_ln_affine:
                g_s = consts.tile([P, INNER], F32)
                nc.sync.dma_start(out=g_s, in_=g_d)
                b_s = consts.tile([P, INNER], F32)
                nc.sync.dma_start(out=b_s, in_=b_d)

            # ================ PASS 1: kv + ksum ================
            with (
                tc.tile_pool(name="p1acc", bufs=1, space="PSUM") as p1acc,
                tc.tile_pool(name="p1work", bufs=1, space="PSUM") as p1work,
            ):
                ktv_ps = [p1acc.tile([P, INNER], F32, tag=f"ktv{c}",
                                     name=f"ktv{c}")
                          for c in range(NCH)]
                ks_ps = p1acc.tile([1, INNER], F32, tag="ks")

                n_sub = n_pad // P
                for blk in range(nblocks):
                    xt = sb1.tile([P, 2, C], F32R, tag="x")
                    nc.sync.dma_start(
                        out=xt,
                        in_=x_d[blk * NB:(blk + 1) * NB, :]
                        .rearrange("(s p) c -> p s c", p=P).bitcast(F32R),
                    )
                    for sub in range(2):
                        si = blk * 2 + sub
                        xT = sb1.tile([P, NCH, P], F32R, tag="xT")
                        for c in range(NCH):
                            tp = p1work.tile([P, P], F32R, tag="xTt")
                            nc.tensor.transpose(
                                tp, xt[:, sub, c * P:(c + 1) * P], ident)
                            nc.scalar.copy(out=xT[:, c, :], in_=tp)
                        # K = sigmoid(x @ k_w)
                        kp = p1work.tile([P, INNER], F32, tag="K")
                        for c in range(NCH):
                            nc.tensor.matmul(kp, xT[:, c, :], wk_s[:, c, :],
                                             start=(c == 0), stop=(c == NCH - 1))
                        ksb = sb1.tile([P, INNER], F32R, tag="Ksb")
                        nc.scalar.activation(
                            out=ksb, in_=kp,
                            func=mybir.ActivationFunctionType.Sigmoid)
                        # V = x @ v_w
                        vp = p1work.tile([P, INNER], F32, tag="V")
                        for c in range(NCH):
                            nc.tensor.matmul(vp, xT[:, c, :], wv_s[:, c, :],
                                             start=(c == 0), stop=(c == NCH - 1))
                        vsb = sb1.tile([P, INNER], F32R, tag="Vsb")
                        nc.scalar.copy(out=vsb, in_=vp)
                        # K^T V accumulation (full [512, 512], masked later)
                        for cm in range(NCH):
                            nc.tensor.matmul(
                                ktv_ps[cm], ksb[:, cm * P:(cm + 1) * P], vsb,
                                start=(si == 0), stop=(si == n_sub - 1))
                        # ksum accumulation
                        nc.tensor.matmul(ks_ps, ones_p, ksb,
                                         start=(si == 0), stop=(si == n_sub - 1))

                for c in range(NCH):
                    kvstage = sb1.tile([P, INNER], F32, tag="kvstage")
                    nc.scalar.copy(out=kvstage, in_=ktv_ps[c])
                    nc.sync.dma_start(out=cc_in[c * P:(c + 1) * P, :],
                                      in_=kvstage)
                ksstage = sb1.tile([1, INNER], F32, tag="ksstage")
                nc.scalar.copy(out=ksstage, in_=ks_ps)
                nc.sync.dma_start(out=cc_in[INNER:INNER + 1, :], in_=ksstage)

            # ================ AllReduce ================
            if use_collective:
                nc.gpsimd.collective_compute(
                    "AllReduce",
                    mybir.AluOpType.add,
                    replica_groups=[list(range(n_cores))],
                    ins=[cc_in[:, :]],
                    outs=[cc_out[:, :]],
                )
            else:
                nc.sync.dma_start(out=cc_out[:, :], in_=cc_in[:, :])

            # ---- build KVB (masked kv) and KS from reduced buffer ----
            kvb = kvsb.tile([P, NCH, INNER], F32R, tag="kvb")
            kvraw = kvsb.tile([P, NCH, INNER], F32, tag="kvraw")
            nc.sync.dma_start(
                out=kvraw,
                in_=cc_out[0:INNER, :].rearrange("(co ci) j -> ci co j", ci=P))
            for c in range(NCH):
                nc.vector.tensor_mul(kvb[:, c, :], kvraw[:, c, :], mask_s)
            kst = kvsb.tile([P, NCH], F32, tag="kst")
            nc.sync.dma_start(
                out=kst,
                in_=cc_out[INNER, :].rearrange("(co ci) -> ci co", ci=P))
            kst2 = kvsb.tile([P, NCH], F32, tag="kst2")
            nc.vector.tensor_scalar_add(kst2, kst, -ks_corr)
            ks_s = kvsb.tile([P, NCH, H], F32R, tag="ks_s")
            for c in range(NCH):
                nc.vector.tensor_scalar_mul(ks_s[:, c, :], mask_s[:, 0:H],
                                            kst2[:, c:c + 1])

            # ================ PASS 2 ================
            with (
                tc.tile_pool(name="p2a", bufs=2, space="PSUM") as p2a,
                tc.tile_pool(name="p2b", bufs=1, space="PSUM") as p2b,
            ):
                for blk in range(nblocks):
                    xt = sb2.tile([P, 2, C], F32R, tag="x2")
                    nc.sync.dma_start(
                        out=xt,
                        in_=x_d[blk * NB:(blk + 1) * NB, :]
                        .rearrange("(s p) c -> p s c", p=P).bitcast(F32R),
                    )
                    xT = sb2.tile([P, NCH, NB], F32R, tag="xT2")
                    for sub in range(2):
                        for c in range(NCH):
                            tp = p2a.tile([P, P], F32R, tag="tr")
                            nc.tensor.transpose(
                                tp, xt[:, sub, c * P:(c + 1) * P], ident)
                            nc.scalar.copy(
                                out=xT[:, c, sub * P:(sub + 1) * P], in_=tp)
                    # QT[jc] = sigmoid(k_w^T x^T) : [128 j, 256 n]
                    qt = sb2.tile([P, NCH, NB], F32R, tag="qt")
                    for jc in range(NCH):
                        qp = p2a.tile([P, NB], F32, tag="qt")
                        for c in range(NCH):
                            nc.tensor.matmul(
                                qp, wk_s[:, c, jc * P:(jc + 1) * P], xT[:, c, :],
                                start=(c == 0), stop=(c == NCH - 1))
                        nc.scalar.activation(
                            out=qt[:, jc, :], in_=qp,
                            func=mybir.ActivationFunctionType.Sigmoid)

                    for sub in range(2):
                        row0 = blk * NB + sub * P
                        # h + h_b + beta  (bias via rank-1 ones matmul)
                        hp = p2b.tile([P, INNER], F32, tag="h")
                        for c in range(NCH):
                            nc.tensor.matmul(
                                hp, xT[:, c, sub * P:(sub + 1) * P],
                                wh_s[:, c, :], start=(c == 0), stop=False)
                        nc.tensor.matmul(hp, ones_1, hb_s,
                                         start=False, stop=True)
                        # num / den
                        nump = p2b.tile([P, INNER], F32, tag="num")
                        for c in range(NCH):
                            nc.tensor.matmul(
                                nump, qt[:, c, sub * P:(sub + 1) * P],
                                kvb[:, c, :], start=(c == 0), stop=(c == NCH - 1))
                        denp = p2b.tile([P, H], F32, tag="den")
                        for c in range(NCH):
                            nc.tensor.matmul(
                                denp, qt[:, c, sub * P:(sub + 1) * P],
                                ks_s[:, c, :], start=(c == 0), stop=(c == NCH - 1))
                        rec = sb2.tile([P, H], F32, tag="rec")
                        nc.vector.reciprocal(rec, denp)
                        attn = sb2.tile([P, INNER], F32, tag="attn")
                        nc.vector.tensor_mul(attn, nump,
                                             _bcast_free(rec[:, :], D))
                        # layernorm stats
                        st = sb2.tile([P, 6], F32, tag="st")
                        nc.vector.bn_stats(out=st, in_=attn)
                        mv = sb2.tile([P, 2], F32, tag="mv")
                        nc.vector.bn_aggr(out=mv, in_=st)
                        rstd = sb2.tile([P, 1], F32, tag="rstd")
                        nc.scalar.activation(
                            out=rstd, in_=mv[:, 1:2],
                            func=mybir.ActivationFunctionType.Sqrt,
                            bias=eps_t, scale=1.0)
                        nc.vector.reciprocal(rstd, rstd)
                        ln = sb2.tile([P, INNER], F32, tag="ln")
                        nc.vector.tensor_scalar(
                            out=ln, in0=attn, scalar1=mv[:, 0:1], scalar2=rstd,
                            op0=mybir.AluOpType.subtract,
                            op1=mybir.AluOpType.mult)
                        if apply_ln_affine:
                            nc.vector.tensor_mul(ln, ln, g_s)
                            nc.vector.tensor_add(ln, ln, b_s)
                        z = sb2.tile([P, INNER], F32R, tag="z")
                        nc.vector.tensor_mul(z, ln, hp)
                        # z^T then y = relu(z @ out_w + out_b)
                        zT = sb2.tile([P, NCH, P], F32R, tag="zT")
                        for c in range(NCH):
                            tp = p2a.tile([P, P], F32R, tag="tr")
                            nc.tensor.transpose(tp, z[:, c * P:(c + 1) * P],
                                                ident)
                            nc.scalar.copy(out=zT[:, c, :], in_=tp)
                        yp = p2b.tile([P, INNER], F32, tag="y")
                        for c in range(NCH):
                            nc.tensor.matmul(yp, zT[:, c, :], wo_s[:, c, :],
                                             start=(c == 0), stop=(c == NCH - 1))
                        ysb = sb2.tile([P, INNER], F32, tag="ysb")
                        nc.vector.tensor_add(ysb, yp, ob_s)
                        nc.scalar.activation(
                            out=ysb, in_=ysb,
                            func=mybir.ActivationFunctionType.Relu)
                        nc.sync.dma_start(out=y_d[row0:row0 + P, :], in_=ysb)

    nc.compile()
    return nc


_cache = {}


def _get_nc(key, **kw):
    if key not in _cache:
        _cache[key] = build_nc(**kw)
    return _cache[key]


def make_in_maps(x, h_w, h_b, k_w, v_w, ln_g, ln_b, out_w, out_b,
                 n_pad=N_PAD, n_real=N_SHARD, n_cores=N_CORES):
    x = np.asarray(x, np.float32)
    n = x.shape[0]
    assert n == n_real * n_cores
    xp = np.zeros((n_cores, n_pad, C), np.float32)
    xp[:, :n_real, :] = x.reshape(n_cores, n_real, C)
    hb_beta = (np.asarray(h_b, np.float32) + BETA).reshape(1, INNER)
    ob_big = np.tile(np.asarray(out_b, np.float32).reshape(1, INNER), (P, 1))
    mask_big = (np.arange(P)[:, None] % H == np.arange(INNER)[None, :] % H
                ).astype(np.float32)
    common = dict(h_w=np.asarray(h_w, np.float32),
                  k_w=np.asarray(k_w, np.float32),
                  v_w=np.asarray(v_w, np.float32),
                  out_w=np.asarray(out_w, np.float32),
                  hb_beta=hb_beta, ob_big=ob_big, mask_big=mask_big)
    apply_affine = not (np.all(np.asarray(ln_g) == 1.0)
                        and np.all(np.asarray(ln_b) == 0.0))
    if apply_affine:
        common["g_big"] = np.tile(np.asarray(ln_g, np.float32).reshape(1, INNER),
                                  (P, 1))
        common["b_big"] = np.tile(np.asarray(ln_b, np.float32).reshape(1, INNER),
                                  (P, 1))
    return [dict(common, x=xp[i]) for i in range(n_cores)], apply_affine


def kernel(x, h_w, h_b, k_w, v_w, ln_g, ln_b, out_w, out_b):
    in_maps, apply_affine = make_in_maps(
        x, h_w, h_b, k_w, v_w, ln_g, ln_b, out_w, out_b)
    nc = _get_nc(("full", apply_affine), apply_ln_affine=apply_affine)
    res = run_bass_kernel_spmd(nc, in_maps, list(range(N_CORES)))
    y = np.concatenate([res.results[i]["y"][:N_SHARD] for i in range(N_CORES)],
                       axis=0)
    return y.astype(np.float32)
